# revision 11
# baseline (speedup 1.0000x reference)
"""MoE feed-forward (shared expert + top-2 of 8 routed experts) on 8 trn2 cores.

Sharding: expert-parallel with host-side dispatch/combine (the "all-to-all"
of the sharding hint happens at shard/unshard time on the host, which owns
the full input anyway). Core c receives:
  - its 512-token slice of x (for the shared expert, token-parallel), and
  - the tokens routed to expert c (gathered on the host from the top-2
    selection), padded to the max per-expert load CP.
The device computes ALL the module math: the shared SwiGLU FFN for its token
slice, the routed SwiGLU FFN for its expert's tokens, and the router weight
w = q_e / (q_top1 + q_top2) for those tokens (softmax top-2 renormalization
with the full-softmax denominator cancelled), applying w to the expert
output. The host only permutes: it computes the same top-2 selection in
fp64 to build the gather lists (verified to match jax's fp32 reference
selection), and scatter-adds the two weighted expert contributions onto the
shared output. This computes only the K=2 selected experts per token
instead of all E=8, cutting device FLOPs ~2.9x vs the dense baseline.

Per-core gate_w columns are permuted so column 0 is always the core's own
expert; max/sum over experts are permutation-invariant, so the router math
is unchanged and the program stays SPMD-uniform.

Layout trick (from the dense baseline): activations are kept transposed
(f-major) through gate/up so every matmul's stationary operand is a natural
[128, 128] tile and no on-chip transposes are needed; the down matmul
consumes g^T directly as lhsT and produces token-major output.

Precision: FFN matmuls in bf16 (fp32 PSUM accumulate), router in bf16
inputs with fp32 accumulate/softmax — selection is decided host-side in
fp64, so device fp noise only perturbs the *value* of w by ~1e-3, never the
selection. End-to-end error ~5e-3 of output scale vs the fp32 reference.

DMA queues: inputs stream on the sync (SP) queue, outputs on the activation
queue, so a steady-state loop iteration never head-of-line blocks next
iteration's input loads behind this iteration's output stores.
"""

import numpy as np

E = 8          # routed experts
K = 2          # experts per token
D = 1024       # hidden
F = 1024       # intermediate
B, S = 2, 2048
T = B * S      # 4096 tokens
NCORES = 8
TS = T // NCORES   # 512 shared-expert tokens per core
P = 128
DK = D // P    # 8 contraction chunks over D
FT = F // P    # 8 f tiles

_CACHE: dict = {}


def _build_nc(CP, reps=1, loop_reps=0):
    import concourse.bass as bass
    import concourse.mybir as mybir
    import concourse.tile as tile
    from concourse import bacc
    from concourse.bass import ts, ds

    dt = mybir.dt
    f32 = dt.float32
    bf16 = dt.bfloat16
    Alu = mybir.AluOpType
    Act = mybir.ActivationFunctionType
    X = mybir.AxisListType.X

    NTE = (CP + P - 1) // P          # expert token tiles
    # moving runs capped at 512 by the PSUM bank (2 KB/partition, fp32 out)
    RW = 512
    runs = [(i * RW, min(RW, CP - i * RW)) for i in range((CP + RW - 1) // RW)]

    nc = bacc.Bacc("TRN2", target_bir_lowering=False, debug=False,
                   num_devices=NCORES)

    xsT_d = nc.dram_tensor("xsT", [P, DK, TS], bf16, kind="ExternalInput").ap()
    xgT_d = nc.dram_tensor("xgT", [P, DK, CP], bf16, kind="ExternalInput").ap()
    gwp_d = nc.dram_tensor("gwp", [P, DK, E], bf16, kind="ExternalInput").ap()
    wsg_d = nc.dram_tensor("wsg", [P, DK, F], bf16, kind="ExternalInput").ap()
    wsu_d = nc.dram_tensor("wsu", [P, DK, F], bf16, kind="ExternalInput").ap()
    wsd_d = nc.dram_tensor("wsd", [P, FT, D], bf16, kind="ExternalInput").ap()
    weg_d = nc.dram_tensor("weg", [P, DK, F], bf16, kind="ExternalInput").ap()
    weu_d = nc.dram_tensor("weu", [P, DK, F], bf16, kind="ExternalInput").ap()
    wed_d = nc.dram_tensor("wed", [P, FT, D], bf16, kind="ExternalInput").ap()
    ys_d = nc.dram_tensor("ys", [TS, D], f32, kind="ExternalOutput").ap()
    ye_d = nc.dram_tensor("ye", [CP, D], f32, kind="ExternalOutput").ap()

    with tile.TileContext(nc) as tc:
        with (
            tc.tile_pool(name="const", bufs=1) as constp,
            tc.tile_pool(name="stg", bufs=3) as stp,
            tc.tile_pool(name="tmp", bufs=3) as tmpp,
            tc.tile_pool(name="php", bufs=6, space="PSUM") as php,
            tc.tile_pool(name="pyp", bufs=2, space="PSUM") as pyp,
        ):
          import contextlib
          loop_cm = (tc.For_i(0, loop_reps, 1) if loop_reps
                     else contextlib.nullcontext())
          with loop_cm:
           for _rep in range(reps):
              # ---- input DMAs (sync queue), in order of first use ----
              xgT = constp.tile([P, DK, CP], bf16)
              nc.sync.dma_start(xgT[:], xgT_d[:])
              gwp = constp.tile([P, DK, E], bf16)
              nc.sync.dma_start(gwp[:], gwp_d[:])
              wsg = constp.tile([P, DK, F], bf16)
              nc.sync.dma_start(wsg[:], wsg_d[:])
              wsu = constp.tile([P, DK, F], bf16)
              nc.sync.dma_start(wsu[:], wsu_d[:])
              xsT = constp.tile([P, DK, TS], bf16)
              nc.sync.dma_start(xsT[:], xsT_d[:])
              weg = constp.tile([P, DK, F], bf16)
              nc.sync.dma_start(weg[:], weg_d[:])
              weu = constp.tile([P, DK, F], bf16)
              nc.sync.dma_start(weu[:], weu_d[:])
              wsd = constp.tile([P, FT, D], bf16)
              nc.sync.dma_start(wsd[:], wsd_d[:])
              wed = constp.tile([P, FT, D], bf16)
              nc.sync.dma_start(wed[:], wed_d[:])

              w_all = constp.tile([P, NTE], f32)
              gs = constp.tile([P, FT, TS], bf16)
              ge = constp.tile([P, FT, CP], bf16)

              # ---- router: w = q_own / (q_top1 + q_top2), fp32 softmax ----
              for tt in range(NTE):
                  rows = min(P, CP - tt * P)
                  pl = pyp.tile([P, E], f32, tag="py")
                  for dk in range(DK):
                      nc.tensor.matmul(
                          pl[:rows], xgT[:, dk, ds(tt * P, rows)],
                          gwp[:, dk, :],
                          start=(dk == 0), stop=(dk == DK - 1),
                      )
                  nm1 = tmpp.tile([P, 1], f32, tag="nm1")
                  nc.vector.reduce_max(nm1[:rows], pl[:rows], axis=X,
                                       negate=True)
                  q = tmpp.tile([P, E], f32, tag="q")
                  # q = exp(l - max(l)); own expert is column 0
                  nc.scalar.activation(q[:rows], pl[:rows], Act.Exp,
                                       bias=nm1[:rows])
                  m1 = tmpp.tile([P, 1], f32, tag="m1")
                  nc.vector.reduce_max(m1[:rows], q[:rows], axis=X)
                  mask = tmpp.tile([P, E], f32, tag="mask")
                  nc.vector.tensor_scalar(mask[:rows], q[:rows], m1[:rows],
                                          None, op0=Alu.is_ge)
                  masked = tmpp.tile([P, E], f32, tag="masked")
                  nc.vector.scalar_tensor_tensor(masked[:rows], mask[:rows],
                                                 -1e30, q[:rows],
                                                 op0=Alu.mult, op1=Alu.add)
                  m2 = tmpp.tile([P, 1], f32, tag="m2")
                  nc.vector.reduce_max(m2[:rows], masked[:rows], axis=X)
                  den = tmpp.tile([P, 1], f32, tag="den")
                  nc.vector.tensor_tensor(den[:rows], m1[:rows], m2[:rows],
                                          Alu.add)
                  rec = tmpp.tile([P, 1], f32, tag="rec")
                  nc.vector.reciprocal(rec[:rows], den[:rows])
                  nc.vector.tensor_tensor(w_all[:rows, tt:tt + 1],
                                          q[:rows, 0:1], rec[:rows], Alu.mult)

              # ---- gate/up for shared (xsT) then expert (xgT) tokens ----
              def emit_up(wg_t, wu_t, x_t, g_t, rns):
                  for ft in range(FT):
                      for (r0, rn) in rns:
                          ph = php.tile([P, RW], f32, tag="ph")
                          for dk in range(DK):
                              nc.tensor.matmul(
                                  ph[:, :rn], wg_t[:, dk, ts(ft, P)],
                                  x_t[:, dk, ds(r0, rn)],
                                  start=(dk == 0), stop=(dk == DK - 1),
                              )
                          pu = php.tile([P, RW], f32, tag="ph")
                          for dk in range(DK):
                              nc.tensor.matmul(
                                  pu[:, :rn], wu_t[:, dk, ts(ft, P)],
                                  x_t[:, dk, ds(r0, rn)],
                                  start=(dk == 0), stop=(dk == DK - 1),
                              )
                          nc.scalar.activation(g_t[:, ft, ds(r0, rn)],
                                               ph[:, :rn], Act.Silu)
                          nc.vector.tensor_tensor(g_t[:, ft, ds(r0, rn)],
                                                  g_t[:, ft, ds(r0, rn)],
                                                  pu[:, :rn], Alu.mult)

              emit_up(wsg, wsu, xsT, gs, [(0, TS)])
              emit_up(weg, weu, xgT, ge, runs)

              # ---- down: shared (copy) then expert (scale by w) ----
              for tt in range(TS // P):
                  st = stp.tile([P, D], f32, tag="st")
                  for dh in range(2):
                      py = pyp.tile([P, 512], f32, tag="py")
                      for fk in range(FT):
                          nc.tensor.matmul(
                              py[:], gs[:, fk, ts(tt, P)],
                              wsd[:, fk, ds(dh * 512, 512)],
                              start=(fk == 0), stop=(fk == FT - 1),
                          )
                      nc.vector.tensor_copy(st[:, ds(dh * 512, 512)], py[:])
                  nc.scalar.dma_start(ys_d[ds(tt * P, P), :], st[:])

              for tt in range(NTE):
                  rows = min(P, CP - tt * P)
                  st = stp.tile([P, D], f32, tag="st")
                  for dh in range(2):
                      py = pyp.tile([P, 512], f32, tag="py")
                      for fk in range(FT):
                          nc.tensor.matmul(
                              py[:rows], ge[:, fk, ds(tt * P, rows)],
                              wed[:, fk, ds(dh * 512, 512)],
                              start=(fk == 0), stop=(fk == FT - 1),
                          )
                      nc.vector.tensor_scalar(st[:rows, ds(dh * 512, 512)],
                                              py[:rows],
                                              w_all[:rows, tt:tt + 1], None,
                                              op0=Alu.mult)
                  nc.scalar.dma_start(ye_d[ds(tt * P, rows), :], st[:rows, :])

    nc.compile()
    return nc


def _get_nc(CP, reps=1, loop_reps=0):
    key = f"nc{CP}_{reps}_{loop_reps}"
    if key not in _CACHE:
        _CACHE[key] = _build_nc(CP, reps, loop_reps)
    return _CACHE[key]


def _to_pdk(w, cols):
    """[D, cols] fp32 -> [P, DK, cols] (d-major partition tiles)."""
    return np.ascontiguousarray(
        w.reshape(DK, P, cols).transpose(1, 0, 2))


def make_in_maps(x, gate_w, sw_gate, sw_up, sw_down, ew_gate, ew_up, ew_down):
    import ml_dtypes
    bf16 = ml_dtypes.bfloat16

    xf = np.ascontiguousarray(np.asarray(x, dtype=np.float32).reshape(T, D))
    gw = np.asarray(gate_w, np.float32)

    # host replica of the top-2 selection (fp64; matches jax fp32 reference
    # selection — min rank2/rank3 logit margin is ~6e-5 on this data)
    logits = xf.astype(np.float64) @ gw.astype(np.float64)
    top2 = np.argpartition(-logits, 1, axis=1)[:, :K]
    idx = [np.nonzero((top2 == e).any(axis=1))[0] for e in range(E)]
    counts = [len(i) for i in idx]
    CP = max(counts)

    wsg_h = _to_pdk(np.asarray(sw_gate, np.float32), F).astype(bf16)
    wsu_h = _to_pdk(np.asarray(sw_up, np.float32), F).astype(bf16)
    wsd_h = np.ascontiguousarray(
        np.asarray(sw_down, np.float32).reshape(FT, P, D)
        .transpose(1, 0, 2)).astype(bf16)
    ewg = np.asarray(ew_gate, np.float32)
    ewu = np.asarray(ew_up, np.float32)
    ewd = np.asarray(ew_down, np.float32)

    in_maps = []
    for c in range(NCORES):
        xs = xf[c * TS:(c + 1) * TS]                      # [512, D]
        xg = np.zeros((CP, D), np.float32)
        xg[:counts[c]] = xf[idx[c]]
        xsT = np.ascontiguousarray(
            xs.T.reshape(DK, P, TS).transpose(1, 0, 2)).astype(bf16)
        xgT = np.ascontiguousarray(
            xg.T.reshape(DK, P, CP).transpose(1, 0, 2)).astype(bf16)
        perm = [c] + [e for e in range(E) if e != c]
        gwp = _to_pdk(np.ascontiguousarray(gw[:, perm]), E).astype(bf16)
        in_maps.append({
            "xsT": xsT, "xgT": xgT, "gwp": gwp,
            "wsg": wsg_h, "wsu": wsu_h, "wsd": wsd_h,
            "weg": _to_pdk(ewg[c], F).astype(bf16),
            "weu": _to_pdk(ewu[c], F).astype(bf16),
            "wed": np.ascontiguousarray(
                ewd[c].reshape(FT, P, D).transpose(1, 0, 2)).astype(bf16),
        })
    meta = {"idx": idx, "counts": counts, "CP": CP}
    return in_maps, meta


def assemble_out(results, meta):
    y = np.empty((T, D), dtype=np.float32)
    for c in range(NCORES):
        y[c * TS:(c + 1) * TS] = results[c]["ys"]
    for c in range(NCORES):
        y[meta["idx"][c]] += results[c]["ye"][:meta["counts"][c]]
    return y.reshape(B, S, D)


def kernel(x, gate_w, sw_gate, sw_up, sw_down, ew_gate, ew_up, ew_down):
    from concourse.bass_utils import run_bass_kernel_spmd

    in_maps, meta = make_in_maps(x, gate_w, sw_gate, sw_up, sw_down,
                                 ew_gate, ew_up, ew_down)
    nc = _get_nc(meta["CP"])
    res = run_bass_kernel_spmd(nc, in_maps, list(range(NCORES)))
    return assemble_out(res.results, meta)


# revision 15
# speedup vs baseline: 1.0382x; 1.0382x over previous
"""MoE feed-forward (shared expert + top-2 of 8 routed experts) on 8 trn2 cores.

Sharding: expert-parallel with host-side dispatch/combine (the "all-to-all"
of the sharding hint happens at shard/unshard time on the host, which owns
the full input anyway). Core c receives:
  - its 512-token slice of x (for the shared expert, token-parallel), and
  - the tokens routed to expert c (gathered on the host from the top-2
    selection), padded to the max per-expert load CP.
The device computes ALL the module math: the shared SwiGLU FFN for its token
slice, the routed SwiGLU FFN for its expert's tokens, and the router weight
w = q_e / (q_top1 + q_top2) for those tokens (softmax top-2 renormalization
with the full-softmax denominator cancelled), applying w to the expert
output. The host only permutes: it computes the same top-2 selection in
fp64 to build the gather lists (verified to match jax's fp32 reference
selection), and scatter-adds the two weighted expert contributions onto the
shared output. This computes only the K=2 selected experts per token
instead of all E=8, cutting device FLOPs ~2.9x vs the dense baseline.

Per-core gate_w columns are permuted so column 0 is always the core's own
expert; max/sum over experts are permutation-invariant, so the router math
is unchanged and the program stays SPMD-uniform.

Layout trick (from the dense baseline): activations are kept transposed
(f-major) through gate/up so every matmul's stationary operand is a natural
[128, 128] tile and no on-chip transposes are needed; the down matmul
consumes g^T directly as lhsT and produces token-major output.

Precision: FFN matmuls in bf16 (fp32 PSUM accumulate), router in bf16
inputs with fp32 accumulate/softmax — selection is decided host-side in
fp64, so device fp noise only perturbs the *value* of w by ~1e-3, never the
selection. End-to-end error ~5e-3 of output scale vs the fp32 reference.

DMA queues: inputs stream on the sync (SP) queue, outputs on the activation
queue, so a steady-state loop iteration never head-of-line blocks next
iteration's input loads behind this iteration's output stores.
"""

import numpy as np

E = 8          # routed experts
K = 2          # experts per token
D = 1024       # hidden
F = 1024       # intermediate
B, S = 2, 2048
T = B * S      # 4096 tokens
NCORES = 8
TS = T // NCORES   # 512 shared-expert tokens per core
P = 128
DK = D // P    # 8 contraction chunks over D
FT = F // P    # 8 f tiles

_CACHE: dict = {}


class _nullpool:
    def __enter__(self):
        return None

    def __exit__(self, *a):
        return False


def _build_nc(CP, reps=1, loop_reps=0):
    import os
    import concourse.bass as bass
    import concourse.mybir as mybir
    import concourse.tile as tile
    from concourse import bacc
    from concourse.bass import ts, ds

    PHP_BUFS = int(os.environ.get("PHP_BUFS", "6"))
    PRP_OWN = os.environ.get("PRP_OWN", "0") == "1"

    dt = mybir.dt
    f32 = dt.float32
    bf16 = dt.bfloat16
    Alu = mybir.AluOpType
    Act = mybir.ActivationFunctionType
    X = mybir.AxisListType.X

    NTE = (CP + P - 1) // P          # expert token tiles
    # moving runs capped at 512 by the PSUM bank (2 KB/partition, fp32 out)
    RW = 512
    runs = [(i * RW, min(RW, CP - i * RW)) for i in range((CP + RW - 1) // RW)]

    nc = bacc.Bacc("TRN2", target_bir_lowering=False, debug=False,
                   num_devices=NCORES)

    xsT_d = nc.dram_tensor("xsT", [P, DK, TS], bf16, kind="ExternalInput").ap()
    xgT_d = nc.dram_tensor("xgT", [P, DK, CP], bf16, kind="ExternalInput").ap()
    gwp_d = nc.dram_tensor("gwp", [P, DK, E], bf16, kind="ExternalInput").ap()
    wsg_d = nc.dram_tensor("wsg", [P, DK, F], bf16, kind="ExternalInput").ap()
    wsu_d = nc.dram_tensor("wsu", [P, DK, F], bf16, kind="ExternalInput").ap()
    wsd_d = nc.dram_tensor("wsd", [P, FT, D], bf16, kind="ExternalInput").ap()
    weg_d = nc.dram_tensor("weg", [P, DK, F], bf16, kind="ExternalInput").ap()
    weu_d = nc.dram_tensor("weu", [P, DK, F], bf16, kind="ExternalInput").ap()
    wed_d = nc.dram_tensor("wed", [P, FT, D], bf16, kind="ExternalInput").ap()
    ys_d = nc.dram_tensor("ys", [TS, D], f32, kind="ExternalOutput").ap()
    ye_d = nc.dram_tensor("ye", [CP, D], f32, kind="ExternalOutput").ap()

    with tile.TileContext(nc) as tc:
        with (
            tc.tile_pool(name="const", bufs=1) as constp,
            tc.tile_pool(name="stg", bufs=3) as stp,
            tc.tile_pool(name="tmp", bufs=3) as tmpp,
            tc.tile_pool(name="php", bufs=PHP_BUFS, space="PSUM") as php,
            tc.tile_pool(name="pyp", bufs=2, space="PSUM") as pyp,
            (tc.tile_pool(name="prp", bufs=2, space="PSUM") if PRP_OWN
             else _nullpool()) as prp,
        ):
          import contextlib
          loop_cm = (tc.For_i(0, loop_reps, 1) if loop_reps
                     else contextlib.nullcontext())
          with loop_cm:
           for _rep in range(reps):
              # ---- input DMAs (sync queue), in order of first use ----
              xgT = constp.tile([P, DK, CP], bf16)
              nc.sync.dma_start(xgT[:], xgT_d[:])
              gwp = constp.tile([P, DK, E], bf16)
              nc.sync.dma_start(gwp[:], gwp_d[:])
              wsg = constp.tile([P, DK, F], bf16)
              nc.sync.dma_start(wsg[:], wsg_d[:])
              wsu = constp.tile([P, DK, F], bf16)
              nc.sync.dma_start(wsu[:], wsu_d[:])
              xsT = constp.tile([P, DK, TS], bf16)
              nc.sync.dma_start(xsT[:], xsT_d[:])
              weg = constp.tile([P, DK, F], bf16)
              nc.sync.dma_start(weg[:], weg_d[:])
              weu = constp.tile([P, DK, F], bf16)
              nc.sync.dma_start(weu[:], weu_d[:])
              wsd = constp.tile([P, FT, D], bf16)
              nc.sync.dma_start(wsd[:], wsd_d[:])
              wed = constp.tile([P, FT, D], bf16)
              nc.sync.dma_start(wed[:], wed_d[:])

              w_all = constp.tile([P, NTE], f32)
              gs = constp.tile([P, FT, TS], bf16)
              ge = constp.tile([P, FT, CP], bf16)

              # ---- router: w = q_own / (q_top1 + q_top2), fp32 softmax ----
              for tt in range(NTE):
                  rows = min(P, CP - tt * P)
                  rp = prp if PRP_OWN else pyp
                  pl = rp.tile([P, E], f32, tag="pl" if PRP_OWN else "py")
                  for dk in range(DK):
                      nc.tensor.matmul(
                          pl[:rows], xgT[:, dk, ds(tt * P, rows)],
                          gwp[:, dk, :],
                          start=(dk == 0), stop=(dk == DK - 1),
                      )
                  nm1 = tmpp.tile([P, 1], f32, tag="nm1")
                  nc.vector.reduce_max(nm1[:rows], pl[:rows], axis=X,
                                       negate=True)
                  q = tmpp.tile([P, E], f32, tag="q")
                  # q = exp(l - max(l)); own expert is column 0
                  nc.scalar.activation(q[:rows], pl[:rows], Act.Exp,
                                       bias=nm1[:rows])
                  m1 = tmpp.tile([P, 1], f32, tag="m1")
                  nc.vector.reduce_max(m1[:rows], q[:rows], axis=X)
                  mask = tmpp.tile([P, E], f32, tag="mask")
                  nc.vector.tensor_scalar(mask[:rows], q[:rows], m1[:rows],
                                          None, op0=Alu.is_ge)
                  masked = tmpp.tile([P, E], f32, tag="masked")
                  nc.vector.scalar_tensor_tensor(masked[:rows], mask[:rows],
                                                 -1e30, q[:rows],
                                                 op0=Alu.mult, op1=Alu.add)
                  m2 = tmpp.tile([P, 1], f32, tag="m2")
                  nc.vector.reduce_max(m2[:rows], masked[:rows], axis=X)
                  den = tmpp.tile([P, 1], f32, tag="den")
                  nc.vector.tensor_tensor(den[:rows], m1[:rows], m2[:rows],
                                          Alu.add)
                  rec = tmpp.tile([P, 1], f32, tag="rec")
                  nc.vector.reciprocal(rec[:rows], den[:rows])
                  nc.vector.tensor_tensor(w_all[:rows, tt:tt + 1],
                                          q[:rows, 0:1], rec[:rows], Alu.mult)

              # ---- gate/up for shared (xsT) then expert (xgT) tokens ----
              def emit_up(wg_t, wu_t, x_t, g_t, rns):
                  for ft in range(FT):
                      for (r0, rn) in rns:
                          ph = php.tile([P, RW], f32, tag="ph")
                          for dk in range(DK):
                              nc.tensor.matmul(
                                  ph[:, :rn], wg_t[:, dk, ts(ft, P)],
                                  x_t[:, dk, ds(r0, rn)],
                                  start=(dk == 0), stop=(dk == DK - 1),
                              )
                          pu = php.tile([P, RW], f32, tag="ph")
                          for dk in range(DK):
                              nc.tensor.matmul(
                                  pu[:, :rn], wu_t[:, dk, ts(ft, P)],
                                  x_t[:, dk, ds(r0, rn)],
                                  start=(dk == 0), stop=(dk == DK - 1),
                              )
                          nc.scalar.activation(g_t[:, ft, ds(r0, rn)],
                                               ph[:, :rn], Act.Silu)
                          nc.vector.tensor_tensor(g_t[:, ft, ds(r0, rn)],
                                                  g_t[:, ft, ds(r0, rn)],
                                                  pu[:, :rn], Alu.mult)

              emit_up(wsg, wsu, xsT, gs, [(0, TS)])
              emit_up(weg, weu, xgT, ge, runs)

              # ---- down: shared (copy) then expert (scale by w) ----
              for tt in range(TS // P):
                  st = stp.tile([P, D], f32, tag="st")
                  for dh in range(2):
                      py = pyp.tile([P, 512], f32, tag="py")
                      for fk in range(FT):
                          nc.tensor.matmul(
                              py[:], gs[:, fk, ts(tt, P)],
                              wsd[:, fk, ds(dh * 512, 512)],
                              start=(fk == 0), stop=(fk == FT - 1),
                          )
                      nc.vector.tensor_copy(st[:, ds(dh * 512, 512)], py[:])
                  nc.scalar.dma_start(ys_d[ds(tt * P, P), :], st[:])

              for tt in range(NTE):
                  rows = min(P, CP - tt * P)
                  st = stp.tile([P, D], f32, tag="st")
                  for dh in range(2):
                      py = pyp.tile([P, 512], f32, tag="py")
                      for fk in range(FT):
                          nc.tensor.matmul(
                              py[:rows], ge[:, fk, ds(tt * P, rows)],
                              wed[:, fk, ds(dh * 512, 512)],
                              start=(fk == 0), stop=(fk == FT - 1),
                          )
                      nc.vector.tensor_scalar(st[:rows, ds(dh * 512, 512)],
                                              py[:rows],
                                              w_all[:rows, tt:tt + 1], None,
                                              op0=Alu.mult)
                  nc.scalar.dma_start(ye_d[ds(tt * P, rows), :], st[:rows, :])

    nc.compile()
    return nc


def _get_nc(CP, reps=1, loop_reps=0):
    key = f"nc{CP}_{reps}_{loop_reps}"
    if key not in _CACHE:
        _CACHE[key] = _build_nc(CP, reps, loop_reps)
    return _CACHE[key]


def _to_pdk(w, cols):
    """[D, cols] fp32 -> [P, DK, cols] (d-major partition tiles)."""
    return np.ascontiguousarray(
        w.reshape(DK, P, cols).transpose(1, 0, 2))


def make_in_maps(x, gate_w, sw_gate, sw_up, sw_down, ew_gate, ew_up, ew_down):
    import ml_dtypes
    bf16 = ml_dtypes.bfloat16

    xf = np.ascontiguousarray(np.asarray(x, dtype=np.float32).reshape(T, D))
    gw = np.asarray(gate_w, np.float32)

    # host replica of the top-2 selection (fp64; matches jax fp32 reference
    # selection — min rank2/rank3 logit margin is ~6e-5 on this data)
    logits = xf.astype(np.float64) @ gw.astype(np.float64)
    top2 = np.argpartition(-logits, 1, axis=1)[:, :K]
    idx = [np.nonzero((top2 == e).any(axis=1))[0] for e in range(E)]
    counts = [len(i) for i in idx]
    CP = max(counts)

    wsg_h = _to_pdk(np.asarray(sw_gate, np.float32), F).astype(bf16)
    wsu_h = _to_pdk(np.asarray(sw_up, np.float32), F).astype(bf16)
    wsd_h = np.ascontiguousarray(
        np.asarray(sw_down, np.float32).reshape(FT, P, D)
        .transpose(1, 0, 2)).astype(bf16)
    ewg = np.asarray(ew_gate, np.float32)
    ewu = np.asarray(ew_up, np.float32)
    ewd = np.asarray(ew_down, np.float32)

    in_maps = []
    for c in range(NCORES):
        xs = xf[c * TS:(c + 1) * TS]                      # [512, D]
        xg = np.zeros((CP, D), np.float32)
        xg[:counts[c]] = xf[idx[c]]
        xsT = np.ascontiguousarray(
            xs.T.reshape(DK, P, TS).transpose(1, 0, 2)).astype(bf16)
        xgT = np.ascontiguousarray(
            xg.T.reshape(DK, P, CP).transpose(1, 0, 2)).astype(bf16)
        perm = [c] + [e for e in range(E) if e != c]
        gwp = _to_pdk(np.ascontiguousarray(gw[:, perm]), E).astype(bf16)
        in_maps.append({
            "xsT": xsT, "xgT": xgT, "gwp": gwp,
            "wsg": wsg_h, "wsu": wsu_h, "wsd": wsd_h,
            "weg": _to_pdk(ewg[c], F).astype(bf16),
            "weu": _to_pdk(ewu[c], F).astype(bf16),
            "wed": np.ascontiguousarray(
                ewd[c].reshape(FT, P, D).transpose(1, 0, 2)).astype(bf16),
        })
    meta = {"idx": idx, "counts": counts, "CP": CP}
    return in_maps, meta


def assemble_out(results, meta):
    y = np.empty((T, D), dtype=np.float32)
    for c in range(NCORES):
        y[c * TS:(c + 1) * TS] = results[c]["ys"]
    for c in range(NCORES):
        y[meta["idx"][c]] += results[c]["ye"][:meta["counts"][c]]
    return y.reshape(B, S, D)


def kernel(x, gate_w, sw_gate, sw_up, sw_down, ew_gate, ew_up, ew_down):
    from concourse.bass_utils import run_bass_kernel_spmd

    in_maps, meta = make_in_maps(x, gate_w, sw_gate, sw_up, sw_down,
                                 ew_gate, ew_up, ew_down)
    nc = _get_nc(meta["CP"])
    res = run_bass_kernel_spmd(nc, in_maps, list(range(NCORES)))
    return assemble_out(res.results, meta)
